# revision 13
# baseline (speedup 1.0000x reference)
"""Multi-head attention (B=4, S=2048, D=1024, H=16, causal) on 8 trn2 cores.

Sharding: core c -> (batch b = c//2, head-half g = c%2, heads g*8..g*8+8).
Each core computes QKV projections for its 8 heads, causal attention, and a
partial dense projection (its 512 input dims). Host sums core pairs + bias.

On-chip layout (per core), all matmuls bf16 with fp32 PSUM accumulate:
  q/k/v     host-prepacked [(chunk,t,p), m] so chunk DMAs are contiguous
  QT/KT     [m, s]   m = 8*64 head dims on partitions (4 chunks of 128),
                     split per 1024-col s-chunk; Q/K bias folded into the
                     PSUM eviction via per-partition tensor_scalar_add
  VA        [s, 8*65] per 128-row s-tile; col 64 of each 65-group = ones
                     (gives softmax denominators via the P@V matmul)
  attention: ST = K_h @ Q_h^T -> [s_k part, s_q free] (two heads row-packed
             via tile_position); causal mask folded into the ST accumulation
             as one extra matmul (maskA^T @ eye adds -1e9 above the
             diagonal); exp on ACT (scale=1/8, no max-subtraction)
  O psum    [65, s_q]: rows 0:64 = unnormalized O^T, row 64 = softmax sums;
             normalize = reciprocal_approx_fast + gpsimd partition_broadcast
             + DVE mul (no DRAM round trip)
  dense     out[s, 1024] partial = O^T.T @ dwT, evicted fp32
"""

import sys

sys.path.insert(0, "/opt/trn_rl_repo")

import numpy as np
import ml_dtypes

import concourse.bass as bass
import concourse.mybir as mybir
import concourse.tile as tile
from concourse.bass_utils import run_bass_kernel_spmd

BF16 = mybir.dt.bfloat16
F32 = mybir.dt.float32
bf16 = ml_dtypes.bfloat16

B, S, D, H, DEPTH = 4, 2048, 1024, 16, 64
NCORES = 8
HPC = H // 2  # 8 heads per core
M = HPC * DEPTH  # 512 head dims per core
CH = 1024  # attention s_q chunk width
NCH = S // CH  # 2
NKT = S // 128  # 16 k tiles
NEG = -1.0e9
EXPF = mybir.ActivationFunctionType.Exp

_CACHE = {}


def _proj_dma(tc, sc, xch, tensors):
    """Emit the input DMAs for one 512-wide s-chunk."""
    nc = tc.nc
    (qr, kr, vr) = tensors[0:3]
    q_ch = xch.tile([128, 8, 512], BF16, tag="q_ch", name=f"q_ch{sc}")
    k_ch = xch.tile([128, 8, 512], BF16, tag="k_ch", name=f"k_ch{sc}")
    v_ch = xch.tile([128, 8, 512], BF16, tag="v_ch", name=f"v_ch{sc}")
    nc.sync.dma_start(out=q_ch, in_=qr[:, sc])
    nc.sync.dma_start(out=k_ch, in_=kr[:, sc])
    nc.sync.dma_start(out=v_ch, in_=vr[:, sc])
    return q_ch, k_ch, v_ch


def _proj_chunk(tc, sc, chtiles, pjps, tensors):
    """Emit projection compute for one 512-wide s-chunk."""
    nc = tc.nc
    (qr, kr, vr, wq_sb, wk_sb, wv_sb, bq_sb, bk_sb, bvb_sb, QT, KT, VA) = tensors
    c = sc // 2
    csl = bass.ts(sc % 2, 512)
    q_ch, k_ch, v_ch = chtiles
    for mt in range(4):
        msl = bass.ts(mt, 128)
        ps_q = pjps.tile([128, 512], F32, tag="ST", name=f"psq{sc}_{mt}")
        ps_k = pjps.tile([128, 512], F32, tag="ST", name=f"psk{sc}_{mt}")
        for t in range(8):
            nc.tensor.matmul(
                ps_q, wq_sb[:, t, msl], q_ch[:, t, :], start=(t == 0), stop=(t == 7)
            )
        nc.vector.tensor_scalar_add(QT[(mt, c)][:, csl], ps_q, bq_sb[:, mt : mt + 1])
        for t in range(8):
            nc.tensor.matmul(
                ps_k, wk_sb[:, t, msl], k_ch[:, t, :], start=(t == 0), stop=(t == 7)
            )
        nc.vector.tensor_scalar_add(KT[(mt, c)][:, csl], ps_k, bk_sb[:, mt : mt + 1])
    for sti in range(4):  # V: [s part, m free]
        st = sc * 4 + sti
        ps_v = pjps.tile([128, 512], F32, tag="ST", name=f"psv{st}")
        for t in range(8):
            nc.tensor.matmul(
                ps_v, v_ch[:, t, bass.ts(sti, 128)], wv_sb[:, t, :],
                start=(t == 0), stop=(t == 7),
            )
        nc.vector.tensor_add(
            VA[st].rearrange("p (h c) -> p h c", c=65)[:, :, 0:64],
            ps_v.rearrange("p (h c) -> p h c", c=64),
            bvb_sb.rearrange("p (h c) -> p h c", c=64),
        )


def _attn_unit(tc, c, hp, pools, tensors):
    """Causal attention for s_q chunk c, head pair hp (heads 2hp, 2hp+1)."""
    nc = tc.nc
    stps, ops, epool, rpool, bcpool, tpool = pools
    (QT, KT, VA, OT, maskA_sb, eye_sb, den_all, rb) = tensors
    O1 = ops.tile([128, CH], F32, tag="O1", name=f"O1_{c}_{hp}")
    O2 = ops.tile([128, CH], F32, tag="O2", name=f"O2_{c}_{hp}")
    nkt = 8 * (c + 1)
    last0 = min(nkt - 1, 8 * c + 3)  # last kt writing cols [0:512)
    for kt in range(nkt):
        j = kt - 8 * c
        qoff = 128 * j if j > 0 else 0
        diag_hf = qoff // 512 if j >= 0 else -1
        ST1 = stps.tile([128, CH], F32, tag="ST", name=f"ST1_{c}_{hp}_{kt}")
        ST2 = stps.tile([128, CH], F32, tag="ST", name=f"ST2_{c}_{hp}_{kt}")
        for hf in (0, 1):
            lo, hi = 512 * hf, 512 * (hf + 1)
            if qoff >= hi:
                continue
            off = max(qoff, lo)
            for idx in (0, 1):
                STx = ST1 if idx == 0 else ST2
                bp = idx * 64
                nc.tensor.matmul(
                    STx[:, off:hi],
                    KT[(hp, kt // 8)][bp : bp + 64, bass.ts(kt % 8, 128)],
                    QT[(hp, c)][bp : bp + 64, off:hi],
                    start=True, stop=(hf != diag_hf),
                    tile_position=(bp, 0),
                )
        if diag_hf >= 0:
            # causal mask: add -1e9 strictly above the diagonal of the
            # 128x128 diagonal block, riding the same PSUM accumulation
            dsl = bass.ds(qoff, 128)
            nc.tensor.matmul(ST1[:, dsl], maskA_sb, eye_sb, start=False, stop=True)
            nc.tensor.matmul(ST2[:, dsl], maskA_sb, eye_sb, start=False, stop=True)
        E1 = epool.tile([128, CH], BF16, tag="E1", name=f"E1_{c}_{hp}_{kt}")
        E2 = epool.tile([128, CH], BF16, tag="E2", name=f"E2_{c}_{hp}_{kt}")
        nc.scalar.activation(E1[:, qoff:CH], ST1[:, qoff:CH], EXPF, scale=0.125)
        nc.scalar.activation(E2[:, qoff:CH], ST2[:, qoff:CH], EXPF, scale=0.125)
        for hf in (0, 1):
            lo, hi = 512 * hf, 512 * (hf + 1)
            if qoff >= hi:
                continue
            off = max(qoff, lo)
            lastk = last0 if hf == 0 else nkt - 1
            for idx, Ox, Ex in ((0, O1, E1), (1, O2, E2)):
                h = 2 * hp + idx
                nc.tensor.matmul(
                    Ox[0:65, off:hi],
                    VA[kt][:, h * 65 : (h + 1) * 65],
                    Ex[:, off:hi],
                    start=(kt == 0), stop=(kt == lastk),
                )
    # evict UNNORMALIZED (frees the O psum slot fast); scatter the softmax
    # sums (row 64) into den_t transposed [8 rows -> 8 partitions each] so
    # the batched per-chunk reciprocal runs 128 elems/lane instead of 1024
    den_t = tensors[-2]
    for idx, Ox in ((0, O1), (1, O2)):
        s = hp * 2 + idx
        dr = rpool.tile([1, CH], F32, tag="dr", name=f"dr{c}_{s}")
        nc.vector.tensor_copy(dr, Ox[64:65, :])
        nc.gpsimd.dma_start(out=den_t[8 * s : 8 * s + 8, :], in_=dr)
        if idx == 0:
            nc.scalar.copy(OT[(hp, c)][0:64, :], Ox[0:64, :])
        else:
            tmp = tpool.tile([64, CH], BF16, tag="tmp", name=f"tmp{c}_{s}")
            nc.scalar.copy(tmp, Ox[0:64, :])
            nc.gpsimd.dma_start(out=OT[(hp, c)][64:128, :], in_=tmp)


def _attn_normalize(tc, c, pools, tensors):
    """Batched softmax normalization for all 4 head-pairs of chunk c:
    one [64, 128] reciprocal, DRAM stride-0 broadcast, in-place DVE mul."""
    nc = tc.nc
    stps, ops, epool, rpool, bcpool, tpool = pools
    (QT, KT, VA, OT, maskA_sb, eye_sb, den_t, rb) = tensors
    nc.vector.reciprocal(den_t, den_t)
    rbf = rpool.tile([64, 128], BF16, tag="rbf", name=f"rbf{c}")
    nc.vector.tensor_copy(rbf, den_t)
    nc.gpsimd.dma_start(
        out=rb[:, :].rearrange("s (j m) -> (s j) m", m=128), in_=rbf
    )
    for hp in range(4):
        bc = bcpool.tile([128, CH], BF16, tag="bc", name=f"bc{c}_{hp}")
        for idx in (0, 1):
            src = rb[hp * 2 + idx : hp * 2 + idx + 1, :]
            nc.gpsimd.dma_start(
                out=bc[idx * 64 : (idx + 1) * 64, :],
                in_=bass.AP(tensor=src.tensor, offset=src.offset, ap=[[0, 64], [1, CH]]),
            )
        nc.vector.tensor_mul(OT[(hp, c)], OT[(hp, c)], bc)


def _dense_st(tc, st, dnps, osb, dw_sb, OT, out):
    nc = tc.nc
    c = st // 8
    ssl = bass.ds((st % 8) * 128, 128)
    for nh in range(2):
        ps = dnps.tile([128, 512], F32, tag="ST", name=f"dn{st}_{nh}")
        for mt in range(4):
            nc.tensor.matmul(
                ps, OT[(mt, c)][:, ssl], dw_sb[:, mt, bass.ts(nh, 512)],
                start=(mt == 0), stop=(mt == 3),
            )
        o_sb = osb.tile([128, 512], F32, tag="o_sb", name=f"o_sb{st}_{nh}")
        if st < 8:
            nc.vector.tensor_copy(o_sb, ps)
        else:
            nc.scalar.copy(o_sb, ps)
        nc.sync.dma_start(out=out[bass.ts(st, 128), bass.ts(nh, 512)], in_=o_sb)


def _body(tc):
    nc = tc.nc
    dram = {t.name: t for t in _CACHE["dram"]}
    out = dram["out"]

    # ---- persistent tiles (one bufs=1 pool, distinct tags -> own slots) ----
    import contextlib
    _pc = contextlib.ExitStack()
    persist = _pc.enter_context(tc.tile_pool(name="persist", bufs=1))

    def P(shape, dt, name):
        return persist.tile(shape, dt, tag=name, name=name)

    wq_sb = P([128, 8, M], BF16, "wq_sb")
    wk_sb = P([128, 8, M], BF16, "wk_sb")
    wv_sb = P([128, 8, M], BF16, "wv_sb")
    dw_sb = P([128, 4, D], BF16, "dw_sb")
    bq_sb = P([128, 4], F32, "bq_sb")
    bk_sb = P([128, 4], F32, "bk_sb")
    bvb_sb = P([128, M], F32, "bvb_sb")
    maskA_sb = P([128, 128], BF16, "maskA_sb")
    eye_sb = P([128, 128], BF16, "eye_sb")

    # wq + chunk-0 x DMAs go first (emitted by the first _proj_chunk call)
    # so the first Q matmuls start ~6us in; the rest follows on the queue.
    nc.sync.dma_start(out=wq_sb, in_=dram["wqpk"][:, :, :])
    nc.sync.dma_start(out=bq_sb, in_=dram["bqc"][:, :])

    def _late_dmas():
        nc.sync.dma_start(out=wk_sb, in_=dram["wkpk"][:, :, :])
        nc.sync.dma_start(out=bk_sb, in_=dram["bkc"][:, :])
        nc.sync.dma_start(out=wv_sb, in_=dram["wvpk"][:, :, :])
        nc.sync.dma_start(out=bvb_sb, in_=dram["bvb"][:, :])
        nc.sync.dma_start(out=maskA_sb, in_=dram["maskA"][:, :])
        nc.sync.dma_start(out=eye_sb, in_=dram["eye"][:, :])
        nc.sync.dma_start(out=dw_sb, in_=dram["dwpk"][:, :, :])

    QT = {
        (mt, c): P([128, CH], BF16, f"QT{mt}_{c}")
        for mt in range(4)
        for c in range(NCH)
    }
    KT = {
        (mt, c): P([128, CH], BF16, f"KT{mt}_{c}")
        for mt in range(4)
        for c in range(NCH)
    }
    VA = {st: P([128, HPC * 65], BF16, f"VA{st}") for st in range(NKT)}
    OT = {
        (hp, c): P([128, CH], BF16, f"OT{hp}_{c}")
        for hp in range(4)
        for c in range(NCH)
    }
    for st in range(NKT):
        nc.gpsimd.memset(VA[st], 1.0)

    qr = dram["qpk"][:, :].rearrange("(c t p) m -> p c t m", t=8, p=128)
    kr = dram["kpk"][:, :].rearrange("(c t p) m -> p c t m", t=8, p=128)
    vr = dram["vpk"][:, :].rearrange("(c t p) m -> p c t m", t=8, p=128)
    rb = _CACHE["rb"]
    ptens = (qr, kr, vr, wq_sb, wk_sb, wv_sb, bq_sb, bk_sb, bvb_sb, QT, KT, VA)

    with (
        tc.tile_pool(name="xch", bufs=2) as xch,
        tc.tile_pool(name="stps", bufs=2, space="PSUM") as stps,
        tc.tile_pool(name="ops", bufs=1, space="PSUM") as ops,
        tc.tile_pool(name="epool", bufs=3) as epool,
        tc.tile_pool(name="rpool", bufs=2) as rpool,
        tc.tile_pool(name="bcpool", bufs=2) as bcpool,
        tc.tile_pool(name="tpool", bufs=2) as tpool,
        tc.tile_pool(name="osb", bufs=2) as osb,
        tc.tile_pool(name="dpool", bufs=2) as dpool,
    ):
        apools = (stps, ops, epool, rpool, bcpool, tpool)
        # emission order = Tile priority: attention c0 only needs QT/KT
        # cols [0:1024) (projection chunks 0-1), so emit it before
        # projection chunks 2-3 -- proj PE work then fills attention's
        # ACT-bound gaps instead of serializing ahead of it.
        ch0 = _proj_dma(tc, 0, xch, ptens)
        _late_dmas()
        _proj_chunk(tc, 0, ch0, stps, ptens)
        ch1 = _proj_dma(tc, 1, xch, ptens)
        _proj_chunk(tc, 1, ch1, stps, ptens)
        den0 = dpool.tile([64, 128], F32, tag="den", name="den0")
        atens0 = (QT, KT, VA, OT, maskA_sb, eye_sb, den0, rb)
        for hp in range(4):
            _attn_unit(tc, 0, hp, apools, atens0)
        ch2 = _proj_dma(tc, 2, xch, ptens)
        _proj_chunk(tc, 2, ch2, stps, ptens)
        ch3 = _proj_dma(tc, 3, xch, ptens)
        _proj_chunk(tc, 3, ch3, stps, ptens)
        _attn_normalize(tc, 0, apools, atens0)
        den1 = dpool.tile([64, 128], F32, tag="den", name="den1")
        atens1 = (QT, KT, VA, OT, maskA_sb, eye_sb, den1, rb)
        for hp in range(4):
            _attn_unit(tc, 1, hp, apools, atens1)
        _attn_normalize(tc, 1, apools, atens1)
        for st in range(NKT):
            _dense_st(tc, st, stps, osb, dw_sb, OT, out)
    _pc.close()


def _legalize_dma_waits(nc):
    """Walrus accepts only one sync wait per instruction (EventSemaphore: 2,
    Drain: special-cased). Spill extra waits onto preceding InstEventSemaphore
    ops on the same engine sequencer."""
    for f in nc.m.functions:
        for blk in f.blocks:
            new_insts = []
            for inst in blk.instructions:
                si = getattr(inst, "sync_info", None)
                exempt = isinstance(inst, mybir.InstEventSemaphore)
                if not exempt and si is not None and len(si.on_wait) > 1:
                    waits = list(si.on_wait)
                    extra, keep = waits[:-1], waits[-1:]
                    while extra:
                        chunk, extra = extra[:2], extra[2:]
                        new_insts.append(
                            mybir.InstEventSemaphore(
                                name=nc.get_next_instruction_name(),
                                engine=inst.engine,
                                ins=[],
                                outs=[],
                                sync_info=mybir.SyncInfo(on_wait=chunk, on_update=[]),
                            )
                        )
                    inst.sync_info = mybir.SyncInfo(
                        on_wait=keep, on_update=list(si.on_update)
                    )
                new_insts.append(inst)
            blk.instructions[:] = new_insts


def _build():
    nc = bass.Bass()
    dram = [
        nc.declare_dram_parameter("qpk", [S * 2, 512], BF16, isOutput=False),
        nc.declare_dram_parameter("kpk", [S * 2, 512], BF16, isOutput=False),
        nc.declare_dram_parameter("vpk", [S * 2, 512], BF16, isOutput=False),
        nc.declare_dram_parameter("wqpk", [128, 8, M], BF16, isOutput=False),
        nc.declare_dram_parameter("wkpk", [128, 8, M], BF16, isOutput=False),
        nc.declare_dram_parameter("wvpk", [128, 8, M], BF16, isOutput=False),
        nc.declare_dram_parameter("dwpk", [128, 4, D], BF16, isOutput=False),
        nc.declare_dram_parameter("bqc", [128, 4], F32, isOutput=False),
        nc.declare_dram_parameter("bkc", [128, 4], F32, isOutput=False),
        nc.declare_dram_parameter("bvb", [128, M], F32, isOutput=False),
        nc.declare_dram_parameter("maskA", [128, 128], BF16, isOutput=False),
        nc.declare_dram_parameter("eye", [128, 128], BF16, isOutput=False),
        nc.declare_dram_parameter("out", [S, D], F32, isOutput=True),
    ]
    _CACHE["dram"] = dram
    _CACHE["rb"] = nc.dram_tensor("rb", [8, CH], BF16)
    with tile.TileContext(nc) as tc:
        _body(tc)
    _legalize_dma_waits(nc)
    return nc


def _get_nc():
    if "nc" not in _CACHE:
        _CACHE["nc"] = _build()
    return _CACHE["nc"]


def _pack_x(xT):
    # [D, S] -> [(chunk, t, p), 512] so chunk DMAs read contiguous lines
    return np.ascontiguousarray(
        xT.reshape(8, 128, 4, 512).transpose(2, 0, 1, 3).reshape(S * 2, 512)
    )


def _make_in_maps(q, k, v, wq_w, wq_b, wk_w, wk_b, wv_w, wv_b, dense_w, dense_b):
    q, k, v = (np.asarray(x, np.float32) for x in (q, k, v))
    maskA = (NEG * np.triu(np.ones((128, 128), np.float32), k=1)).astype(bf16)
    eye = np.eye(128, dtype=np.float32).astype(bf16)
    in_maps = []
    for core in range(NCORES):
        b, g = divmod(core, 2)
        hs = slice(g * M, (g + 1) * M)
        wqT = np.asarray(wq_w)[hs].T.astype(bf16)  # [D, M]
        wkT = np.asarray(wk_w)[hs].T.astype(bf16)
        wvT = np.asarray(wv_w)[hs].T.astype(bf16)
        dwT = np.asarray(dense_w)[:, hs].T.astype(bf16)  # [M, D]
        in_maps.append(
            {
                "qpk": _pack_x(q[b].T.astype(bf16)),
                "kpk": _pack_x(k[b].T.astype(bf16)),
                "vpk": _pack_x(v[b].T.astype(bf16)),
                "wqpk": np.ascontiguousarray(
                    wqT.reshape(8, 128, M).transpose(1, 0, 2)
                ),
                "wkpk": np.ascontiguousarray(
                    wkT.reshape(8, 128, M).transpose(1, 0, 2)
                ),
                "wvpk": np.ascontiguousarray(
                    wvT.reshape(8, 128, M).transpose(1, 0, 2)
                ),
                "dwpk": np.ascontiguousarray(
                    dwT.reshape(4, 128, D).transpose(1, 0, 2)
                ),
                "bqc": np.ascontiguousarray(
                    np.asarray(wq_b, np.float32)[hs].reshape(4, 128).T
                ),
                "bkc": np.ascontiguousarray(
                    np.asarray(wk_b, np.float32)[hs].reshape(4, 128).T
                ),
                "bvb": np.ascontiguousarray(
                    np.broadcast_to(np.asarray(wv_b, np.float32)[hs], (128, M))
                ),
                "maskA": maskA,
                "eye": eye,
            }
        )
    return in_maps


def kernel(q, k, v, wq_w, wq_b, wk_w, wk_b, wv_w, wv_b, dense_w, dense_b):
    nc = _get_nc()
    in_maps = _make_in_maps(
        q, k, v, wq_w, wq_b, wk_w, wk_b, wv_w, wv_b, dense_w, dense_b
    )
    res = run_bass_kernel_spmd(nc, in_maps, list(range(NCORES)))
    _CACHE["last_res"] = res
    outs = [r["out"] for r in res.results]
    final = np.empty((B, S, D), np.float32)
    db = np.asarray(dense_b, np.float32)
    for b in range(B):
        final[b] = outs[2 * b] + outs[2 * b + 1] + db[None, :]
    return final


# revision 17
# speedup vs baseline: 1.1780x; 1.1780x over previous
"""Multi-head attention (B=4, S=2048, D=1024, H=16, causal) on 8 trn2 cores.

Sharding: core c -> (batch b = c//2, head-half g = c%2, heads g*8..g*8+8).
Each core computes QKV projections for its 8 heads, causal attention, and a
partial dense projection (its 512 input dims). Host sums core pairs + bias.

On-chip layout (per core), all matmuls bf16 with fp32 PSUM accumulate:
  q/k/v     host-prepacked [(chunk,t,p), m] so chunk DMAs are contiguous
  QT/KT     [m, s]   m = 8*64 head dims on partitions (4 chunks of 128),
                     split per 1024-col s-chunk; Q/K bias folded into the
                     PSUM eviction via per-partition tensor_scalar_add
  VA        [s, 8*65] per 128-row s-tile; col 64 of each 65-group = ones
                     (gives softmax denominators via the P@V matmul)
  attention: ST = K_h @ Q_h^T -> [s_k part, s_q free] (two heads row-packed
             via tile_position); causal mask folded into the ST accumulation
             as one extra matmul (maskA^T @ eye adds -1e9 above the
             diagonal); exp on ACT (scale=1/8, no max-subtraction)
  O psum    [65, s_q]: rows 0:64 = unnormalized O^T, row 64 = softmax sums;
             normalize = reciprocal_approx_fast + gpsimd partition_broadcast
             + DVE mul (no DRAM round trip)
  dense     out[s, 1024] partial = O^T.T @ dwT, evicted fp32
"""

import sys

sys.path.insert(0, "/opt/trn_rl_repo")

import numpy as np
import ml_dtypes

import concourse.bass as bass
import concourse.mybir as mybir
import concourse.tile as tile
from concourse.bass_utils import run_bass_kernel_spmd

BF16 = mybir.dt.bfloat16
F32 = mybir.dt.float32
bf16 = ml_dtypes.bfloat16

B, S, D, H, DEPTH = 4, 2048, 1024, 16, 64
NCORES = 8
HPC = H // 2  # 8 heads per core
M = HPC * DEPTH  # 512 head dims per core
CH = 1024  # attention s_q chunk width
NCH = S // CH  # 2
NKT = S // 128  # 16 k tiles
NEG = -1.0e9
EXPF = mybir.ActivationFunctionType.Exp

_CACHE = {}


def _proj_dma(tc, sc, xch, tensors):
    """Emit the input DMAs for one 512-wide s-chunk."""
    nc = tc.nc
    (qr, kr, vr) = tensors[0:3]
    q_ch = xch.tile([128, 8, 512], BF16, tag="q_ch", name=f"q_ch{sc}")
    k_ch = xch.tile([128, 8, 512], BF16, tag="k_ch", name=f"k_ch{sc}")
    v_ch = xch.tile([128, 8, 512], BF16, tag="v_ch", name=f"v_ch{sc}")
    nc.sync.dma_start(out=q_ch, in_=qr[:, sc])
    nc.sync.dma_start(out=k_ch, in_=kr[:, sc])
    nc.sync.dma_start(out=v_ch, in_=vr[:, sc])
    return q_ch, k_ch, v_ch


def _proj_chunk(tc, sc, chtiles, pjps, tensors):
    """Emit projection compute for one 512-wide s-chunk."""
    nc = tc.nc
    (qr, kr, vr, wq_sb, wk_sb, wv_sb, bq_sb, bk_sb, bvb_sb, QT, KT, VA) = tensors
    c = sc // 2
    csl = bass.ts(sc % 2, 512)
    q_ch, k_ch, v_ch = chtiles
    for mt in range(4):
        msl = bass.ts(mt, 128)
        ps_q = pjps.tile([128, 512], F32, tag="ST", name=f"psq{sc}_{mt}")
        ps_k = pjps.tile([128, 512], F32, tag="ST", name=f"psk{sc}_{mt}")
        for t in range(8):
            nc.tensor.matmul(
                ps_q, wq_sb[:, t, msl], q_ch[:, t, :], start=(t == 0), stop=(t == 7)
            )
        nc.vector.tensor_scalar_add(QT[(mt, c)][:, csl], ps_q, bq_sb[:, mt : mt + 1])
        for t in range(8):
            nc.tensor.matmul(
                ps_k, wk_sb[:, t, msl], k_ch[:, t, :], start=(t == 0), stop=(t == 7)
            )
        nc.vector.tensor_scalar_add(KT[(mt, c)][:, csl], ps_k, bk_sb[:, mt : mt + 1])
    for sti in range(4):  # V: [s part, m free]
        st = sc * 4 + sti
        ps_v = pjps.tile([128, 512], F32, tag="ST", name=f"psv{st}")
        for t in range(8):
            nc.tensor.matmul(
                ps_v, v_ch[:, t, bass.ts(sti, 128)], wv_sb[:, t, :],
                start=(t == 0), stop=(t == 7),
            )
        nc.vector.tensor_add(
            VA[st].rearrange("p (h c) -> p h c", c=65)[:, :, 0:64],
            ps_v.rearrange("p (h c) -> p h c", c=64),
            bvb_sb.rearrange("p (h c) -> p h c", c=64),
        )


def _attn_unit(tc, c, hp, pools, tensors):
    """Causal attention for s_q chunk c, head pair hp (heads 2hp, 2hp+1)."""
    nc = tc.nc
    stps, ops, epool, rpool, bcpool, tpool = pools
    (QT, KT, VA, OT, maskA_sb, den_all, rb) = tensors
    O1 = ops.tile([128, CH], F32, tag="O1", name=f"O1_{c}_{hp}")
    O2 = ops.tile([128, CH], F32, tag="O2", name=f"O2_{c}_{hp}")
    nkt = 8 * (c + 1)
    last0 = min(nkt - 1, 8 * c + 3)  # last kt writing cols [0:512)
    for kt in range(nkt):
        j = kt - 8 * c
        qoff = 128 * j if j > 0 else 0
        diag_hf = qoff // 512 if j >= 0 else -1
        ST1 = stps.tile([128, CH], F32, tag="ST", name=f"ST1_{c}_{hp}_{kt}")
        ST2 = stps.tile([128, CH], F32, tag="ST", name=f"ST2_{c}_{hp}_{kt}")
        for hf in (0, 1):
            lo, hi = 512 * hf, 512 * (hf + 1)
            if qoff >= hi:
                continue
            off = max(qoff, lo)
            for idx in (0, 1):
                STx = ST1 if idx == 0 else ST2
                bp = idx * 64
                nc.tensor.matmul(
                    STx[:, off:hi],
                    KT[(hp, kt // 8)][bp : bp + 64, bass.ts(kt % 8, 128)],
                    QT[(hp, c)][bp : bp + 64, off:hi],
                    start=True, stop=True,
                    tile_position=(bp, 0),
                )
        E1 = epool.tile([128, CH], BF16, tag="E1", name=f"E1_{c}_{hp}_{kt}")
        E2 = epool.tile([128, CH], BF16, tag="E2", name=f"E2_{c}_{hp}_{kt}")
        nc.scalar.activation(E1[:, qoff:CH], ST1[:, qoff:CH], EXPF, scale=0.125)
        nc.scalar.activation(E2[:, qoff:CH], ST2[:, qoff:CH], EXPF, scale=0.125)
        if diag_hf >= 0:
            # causal mask: zero E strictly below the diagonal of the
            # 128x128 diagonal block (multiplicative triu mask), on the
            # otherwise-idle gpsimd engine to keep PE/ACT free
            dsl = bass.ds(qoff, 128)
            nc.gpsimd.tensor_mul(E1[:, dsl], E1[:, dsl], maskA_sb)
            nc.gpsimd.tensor_mul(E2[:, dsl], E2[:, dsl], maskA_sb)
        for hf in (0, 1):
            lo, hi = 512 * hf, 512 * (hf + 1)
            if qoff >= hi:
                continue
            off = max(qoff, lo)
            lastk = last0 if hf == 0 else nkt - 1
            for idx, Ox, Ex in ((0, O1, E1), (1, O2, E2)):
                h = 2 * hp + idx
                nc.tensor.matmul(
                    Ox[0:65, off:hi],
                    VA[kt][:, h * 65 : (h + 1) * 65],
                    Ex[:, off:hi],
                    start=(kt == 0), stop=(kt == lastk),
                )
    # evict UNNORMALIZED (frees the O psum slot fast); scatter the softmax
    # sums (row 64) into den_t transposed [8 rows -> 8 partitions each] so
    # the batched per-chunk reciprocal runs 128 elems/lane instead of 1024
    den_t = tensors[-2]
    for idx, Ox in ((0, O1), (1, O2)):
        s = hp * 2 + idx
        dr = rpool.tile([1, CH], F32, tag="dr", name=f"dr{c}_{s}")
        nc.vector.tensor_copy(dr, Ox[64:65, :])
        nc.gpsimd.dma_start(out=den_t[8 * s : 8 * s + 8, :], in_=dr)
        if idx == 0:
            nc.vector.tensor_copy(OT[(hp, c)][0:64, :], Ox[0:64, :])
        else:
            tmp = tpool.tile([64, CH], BF16, tag="tmp", name=f"tmp{c}_{s}")
            nc.vector.tensor_copy(tmp, Ox[0:64, :])
            nc.gpsimd.dma_start(out=OT[(hp, c)][64:128, :], in_=tmp)


def _attn_normalize(tc, c, pools, tensors):
    """Batched softmax normalization for all 4 head-pairs of chunk c:
    one [64, 128] reciprocal, DRAM stride-0 broadcast, in-place DVE mul."""
    nc = tc.nc
    stps, ops, epool, rpool, bcpool, tpool = pools
    (QT, KT, VA, OT, maskA_sb, den_t, rb) = tensors
    nc.vector.reciprocal(den_t, den_t)
    rbf = rpool.tile([64, 128], BF16, tag="rbf", name=f"rbf{c}")
    nc.vector.tensor_copy(rbf, den_t)
    nc.gpsimd.dma_start(
        out=rb[:, :].rearrange("s (j m) -> (s j) m", m=128), in_=rbf
    )
    for hp in range(4):
        bc = bcpool.tile([128, CH], BF16, tag="bc", name=f"bc{c}_{hp}")
        for idx in (0, 1):
            src = rb[hp * 2 + idx : hp * 2 + idx + 1, :]
            nc.gpsimd.dma_start(
                out=bc[idx * 64 : (idx + 1) * 64, :],
                in_=bass.AP(tensor=src.tensor, offset=src.offset, ap=[[0, 64], [1, CH]]),
            )
        nc.vector.tensor_mul(OT[(hp, c)], OT[(hp, c)], bc)


def _dense_st(tc, st, dnps, osb, dw_sb, OT, out):
    nc = tc.nc
    c = st // 8
    ssl = bass.ds((st % 8) * 128, 128)
    for nh in range(2):
        ps = dnps.tile([128, 512], F32, tag="ST", name=f"dn{st}_{nh}")
        for mt in range(4):
            nc.tensor.matmul(
                ps, OT[(mt, c)][:, ssl], dw_sb[:, mt, bass.ts(nh, 512)],
                start=(mt == 0), stop=(mt == 3),
            )
        o_sb = osb.tile([128, 512], F32, tag="o_sb", name=f"o_sb{st}_{nh}")
        if st < 8:
            nc.vector.tensor_copy(o_sb, ps)
        else:
            nc.scalar.copy(o_sb, ps)
        nc.sync.dma_start(out=out[bass.ts(st, 128), bass.ts(nh, 512)], in_=o_sb)


def _body(tc):
    nc = tc.nc
    dram = {t.name: t for t in _CACHE["dram"]}
    out = dram["out"]

    # ---- persistent tiles (one bufs=1 pool, distinct tags -> own slots) ----
    import contextlib
    _pc = contextlib.ExitStack()
    persist = _pc.enter_context(tc.tile_pool(name="persist", bufs=1))

    def P(shape, dt, name):
        return persist.tile(shape, dt, tag=name, name=name)

    wq_sb = P([128, 8, M], BF16, "wq_sb")
    wk_sb = P([128, 8, M], BF16, "wk_sb")
    wv_sb = P([128, 8, M], BF16, "wv_sb")
    dw_sb = P([128, 4, D], BF16, "dw_sb")
    bq_sb = P([128, 4], F32, "bq_sb")
    bk_sb = P([128, 4], F32, "bk_sb")
    bvb_sb = P([128, M], F32, "bvb_sb")
    maskA_sb = P([128, 128], BF16, "maskA_sb")

    # wq + chunk-0 x DMAs go first (emitted by the first _proj_chunk call)
    # so the first Q matmuls start ~6us in; the rest follows on the queue.
    nc.sync.dma_start(out=wq_sb, in_=dram["wqpk"][:, :, :])
    nc.sync.dma_start(out=bq_sb, in_=dram["bqc"][:, :])

    def _late_dmas():
        nc.sync.dma_start(out=wk_sb, in_=dram["wkpk"][:, :, :])
        nc.sync.dma_start(out=bk_sb, in_=dram["bkc"][:, :])
        nc.sync.dma_start(out=wv_sb, in_=dram["wvpk"][:, :, :])
        nc.sync.dma_start(out=bvb_sb, in_=dram["bvb"][:, :])
        nc.sync.dma_start(out=maskA_sb, in_=dram["maskA"][:, :])
        nc.sync.dma_start(out=dw_sb, in_=dram["dwpk"][:, :, :])

    QT = {
        (mt, c): P([128, CH], BF16, f"QT{mt}_{c}")
        for mt in range(4)
        for c in range(NCH)
    }
    KT = {
        (mt, c): P([128, CH], BF16, f"KT{mt}_{c}")
        for mt in range(4)
        for c in range(NCH)
    }
    VA = {st: P([128, HPC * 65], BF16, f"VA{st}") for st in range(NKT)}
    OT = {
        (hp, c): P([128, CH], BF16, f"OT{hp}_{c}")
        for hp in range(4)
        for c in range(NCH)
    }
    for st in range(NKT):
        nc.gpsimd.memset(VA[st], 1.0)

    qr = dram["qpk"][:, :].rearrange("(c t p) m -> p c t m", t=8, p=128)
    kr = dram["kpk"][:, :].rearrange("(c t p) m -> p c t m", t=8, p=128)
    vr = dram["vpk"][:, :].rearrange("(c t p) m -> p c t m", t=8, p=128)
    rb = _CACHE["rb"]
    ptens = (qr, kr, vr, wq_sb, wk_sb, wv_sb, bq_sb, bk_sb, bvb_sb, QT, KT, VA)

    with (
        tc.tile_pool(name="xch", bufs=2) as xch,
        tc.tile_pool(name="stps", bufs=2, space="PSUM") as stps,
        tc.tile_pool(name="ops", bufs=1, space="PSUM") as ops,
        tc.tile_pool(name="epool", bufs=3) as epool,
        tc.tile_pool(name="rpool", bufs=2) as rpool,
        tc.tile_pool(name="bcpool", bufs=2) as bcpool,
        tc.tile_pool(name="tpool", bufs=2) as tpool,
        tc.tile_pool(name="osb", bufs=2) as osb,
        tc.tile_pool(name="dpool", bufs=2) as dpool,
    ):
        apools = (stps, ops, epool, rpool, bcpool, tpool)
        # emission order = Tile priority: attention c0 only needs QT/KT
        # cols [0:1024) (projection chunks 0-1), so emit it before
        # projection chunks 2-3 -- proj PE work then fills attention's
        # ACT-bound gaps instead of serializing ahead of it.
        ch0 = _proj_dma(tc, 0, xch, ptens)
        _late_dmas()
        _proj_chunk(tc, 0, ch0, stps, ptens)
        ch1 = _proj_dma(tc, 1, xch, ptens)
        _proj_chunk(tc, 1, ch1, stps, ptens)
        den0 = dpool.tile([64, 128], F32, tag="den", name="den0")
        atens0 = (QT, KT, VA, OT, maskA_sb, den0, rb)
        for hp in range(4):
            _attn_unit(tc, 0, hp, apools, atens0)
        ch2 = _proj_dma(tc, 2, xch, ptens)
        _proj_chunk(tc, 2, ch2, stps, ptens)
        ch3 = _proj_dma(tc, 3, xch, ptens)
        _proj_chunk(tc, 3, ch3, stps, ptens)
        _attn_normalize(tc, 0, apools, atens0)
        den1 = dpool.tile([64, 128], F32, tag="den", name="den1")
        atens1 = (QT, KT, VA, OT, maskA_sb, den1, rb)
        for hp in range(4):
            _attn_unit(tc, 1, hp, apools, atens1)
        _attn_normalize(tc, 1, apools, atens1)
        for st in range(NKT):
            _dense_st(tc, st, stps, osb, dw_sb, OT, out)
    _pc.close()


def _legalize_dma_waits(nc):
    """Walrus accepts only one sync wait per instruction (EventSemaphore: 2,
    Drain: special-cased). Spill extra waits onto preceding InstEventSemaphore
    ops on the same engine sequencer."""
    for f in nc.m.functions:
        for blk in f.blocks:
            new_insts = []
            for inst in blk.instructions:
                si = getattr(inst, "sync_info", None)
                exempt = isinstance(inst, mybir.InstEventSemaphore)
                if not exempt and si is not None and len(si.on_wait) > 1:
                    waits = list(si.on_wait)
                    extra, keep = waits[:-1], waits[-1:]
                    while extra:
                        chunk, extra = extra[:2], extra[2:]
                        new_insts.append(
                            mybir.InstEventSemaphore(
                                name=nc.get_next_instruction_name(),
                                engine=inst.engine,
                                ins=[],
                                outs=[],
                                sync_info=mybir.SyncInfo(on_wait=chunk, on_update=[]),
                            )
                        )
                    inst.sync_info = mybir.SyncInfo(
                        on_wait=keep, on_update=list(si.on_update)
                    )
                new_insts.append(inst)
            blk.instructions[:] = new_insts


def _build():
    nc = bass.Bass()
    dram = [
        nc.declare_dram_parameter("qpk", [S * 2, 512], BF16, isOutput=False),
        nc.declare_dram_parameter("kpk", [S * 2, 512], BF16, isOutput=False),
        nc.declare_dram_parameter("vpk", [S * 2, 512], BF16, isOutput=False),
        nc.declare_dram_parameter("wqpk", [128, 8, M], BF16, isOutput=False),
        nc.declare_dram_parameter("wkpk", [128, 8, M], BF16, isOutput=False),
        nc.declare_dram_parameter("wvpk", [128, 8, M], BF16, isOutput=False),
        nc.declare_dram_parameter("dwpk", [128, 4, D], BF16, isOutput=False),
        nc.declare_dram_parameter("bqc", [128, 4], F32, isOutput=False),
        nc.declare_dram_parameter("bkc", [128, 4], F32, isOutput=False),
        nc.declare_dram_parameter("bvb", [128, M], F32, isOutput=False),
        nc.declare_dram_parameter("maskA", [128, 128], BF16, isOutput=False),
        nc.declare_dram_parameter("out", [S, D], F32, isOutput=True),
    ]
    _CACHE["dram"] = dram
    _CACHE["rb"] = nc.dram_tensor("rb", [8, CH], BF16)
    with tile.TileContext(nc) as tc:
        _body(tc)
    _legalize_dma_waits(nc)
    return nc


def _get_nc():
    if "nc" not in _CACHE:
        _CACHE["nc"] = _build()
    return _CACHE["nc"]


def _pack_x(xT):
    # [D, S] -> [(chunk, t, p), 512] so chunk DMAs read contiguous lines
    return np.ascontiguousarray(
        xT.reshape(8, 128, 4, 512).transpose(2, 0, 1, 3).reshape(S * 2, 512)
    )


def _make_in_maps(q, k, v, wq_w, wq_b, wk_w, wk_b, wv_w, wv_b, dense_w, dense_b):
    q, k, v = (np.asarray(x, np.float32) for x in (q, k, v))
    maskA = np.triu(np.ones((128, 128), np.float32)).astype(bf16)
    in_maps = []
    for core in range(NCORES):
        b, g = divmod(core, 2)
        hs = slice(g * M, (g + 1) * M)
        wqT = np.asarray(wq_w)[hs].T.astype(bf16)  # [D, M]
        wkT = np.asarray(wk_w)[hs].T.astype(bf16)
        wvT = np.asarray(wv_w)[hs].T.astype(bf16)
        dwT = np.asarray(dense_w)[:, hs].T.astype(bf16)  # [M, D]
        in_maps.append(
            {
                "qpk": _pack_x(q[b].T.astype(bf16)),
                "kpk": _pack_x(k[b].T.astype(bf16)),
                "vpk": _pack_x(v[b].T.astype(bf16)),
                "wqpk": np.ascontiguousarray(
                    wqT.reshape(8, 128, M).transpose(1, 0, 2)
                ),
                "wkpk": np.ascontiguousarray(
                    wkT.reshape(8, 128, M).transpose(1, 0, 2)
                ),
                "wvpk": np.ascontiguousarray(
                    wvT.reshape(8, 128, M).transpose(1, 0, 2)
                ),
                "dwpk": np.ascontiguousarray(
                    dwT.reshape(4, 128, D).transpose(1, 0, 2)
                ),
                "bqc": np.ascontiguousarray(
                    np.asarray(wq_b, np.float32)[hs].reshape(4, 128).T
                ),
                "bkc": np.ascontiguousarray(
                    np.asarray(wk_b, np.float32)[hs].reshape(4, 128).T
                ),
                "bvb": np.ascontiguousarray(
                    np.broadcast_to(np.asarray(wv_b, np.float32)[hs], (128, M))
                ),
                "maskA": maskA,
            }
        )
    return in_maps


def kernel(q, k, v, wq_w, wq_b, wk_w, wk_b, wv_w, wv_b, dense_w, dense_b):
    nc = _get_nc()
    in_maps = _make_in_maps(
        q, k, v, wq_w, wq_b, wk_w, wk_b, wv_w, wv_b, dense_w, dense_b
    )
    res = run_bass_kernel_spmd(nc, in_maps, list(range(NCORES)))
    _CACHE["last_res"] = res
    outs = [r["out"] for r in res.results]
    final = np.empty((B, S, D), np.float32)
    db = np.asarray(dense_b, np.float32)
    for b in range(B):
        final[b] = outs[2 * b] + outs[2 * b + 1] + db[None, :]
    return final
